# revision 34
# baseline (speedup 1.0000x reference)
"""Causal self-attention (RoPE) Trainium2 Bass kernel.

Problem: B=4, T=2048, C=1024, H=16 heads, D=64, fp32 I/O.
Sharding: 8 cores = 4 (batch) x 2 (head-group TP). Each core computes
qkv/attention/proj for 1 batch and 8 heads, producing a partial
projection output; the host sums the two TP partials per batch.

Per-core pipeline (chunk-interleaved to overlap PE matmuls with the
ACT-engine exp of the softmax):
  stage A (per 512-token chunk c): qkv projection + RoPE -> q_rot(c)
    (transient), k_rot[:, c] (persistent), v(c) (persistent, with an
    appended ones column for the softmax denominator)
  stage B (per chunk c, head-pair p, key-block jb): one [128, 1024]
    psum tile holds both heads' S_T scores for key block jb; the two
    K=64 QK matmuls are adjacent with different row groups so they
    run concurrently in the PE array.  One exp on ACT covers both
    heads (2-range AP when the causal io-shrink applies), a [128,128]
    staircase mask on DVE for diagonal blocks, then per-head
    out_T = v_ext^T @ P_T accumulated over key blocks; row 64 of the
    accumulator is the softmax denominator l.  Normalize with
    reciprocal (read straight from PSUM) + gpsimd partition_broadcast.
  proj (per chunk): y[chunk] = o_T^T @ WprojT, partial over this
    core's 512 input features.
"""

import numpy as np
from contextlib import ExitStack

import concourse.bacc as bacc
import concourse.bass as bass
import concourse.mybir as mybir
import concourse.tile as tile

# ---------------- constants ----------------
B = 4
T = 2048
C = 1024
H = 16
D = 64
L = 8  # local heads per core
NCORES = 8
ROPE_BASE = 10000.0

CH = 512  # t-chunk size
NCH = T // CH  # 4 chunks
KT = C // 128  # 8 contraction tiles
NP = L // 2  # 4 head-pair tiles
SCALE = 1.0 / np.sqrt(D)

F32 = mybir.dt.float32
BF16 = mybir.dt.bfloat16

# matmul operand dtypes
DT_X = BF16  # x / Wqkv / Wv operands
DT_K = BF16  # q_rot / k_rot
DT_PV = BF16  # P tiles, v tiles, masks
DT_O = BF16  # o_T tiles / WprojT


def _np_dt(dt):
    return mybir.dt.np(dt)


# ---------------- device kernel ----------------


def attn_body(ctx: ExitStack, tc: tile.TileContext, outs, ins):
    """outs = (y [T, C] f32,); ins = (xt4, wqk, wv, wp, cs4, sn4, stair)."""
    nc = tc.nc
    (y,) = outs if isinstance(outs, (tuple, list)) else (outs,)
    xt4, wqk, wv, wp, cs4, sn4, stair = ins

    TB = T // 128  # 16 key blocks

    consts = ctx.enter_context(tc.tile_pool(name="consts", bufs=1))
    xpool = ctx.enter_context(tc.tile_pool(name="xpool", bufs=16))
    cspool = ctx.enter_context(tc.tile_pool(name="cspool", bufs=4))
    qrpool = ctx.enter_context(tc.tile_pool(name="qrpool", bufs=8))
    rtmp = ctx.enter_context(tc.tile_pool(name="rtmp", bufs=4))
    ptpool = ctx.enter_context(tc.tile_pool(name="ptpool", bufs=6))
    otpool = ctx.enter_context(tc.tile_pool(name="otpool", bufs=16))
    yepool = ctx.enter_context(tc.tile_pool(name="yepool", bufs=3))
    lpool = ctx.enter_context(tc.tile_pool(name="lpool", bufs=4))
    pmisc = ctx.enter_context(tc.tile_pool(name="pmisc", bufs=2, space="PSUM"))
    pss_pool = ctx.enter_context(tc.tile_pool(name="pss", bufs=2, space="PSUM"))
    pso_pool = ctx.enter_context(tc.tile_pool(name="pso", bufs=2, space="PSUM"))

    # persistent tiles
    wqk_sb = [consts.tile([128, 2 * 512], DT_X, name=f"wqk{k}") for k in range(KT)]
    wv_sb = [consts.tile([128, 512], DT_X, name=f"wv{k}") for k in range(KT)]
    wp_sb = [consts.tile([128, C], DT_O, name=f"wp{p}") for p in range(NP)]
    stair_sb = consts.tile([128, 2, 128], DT_PV, name="stair")
    k_rot = [consts.tile([128, T], DT_K, name=f"krot{p}") for p in range(NP)]
    v_sb = consts.tile([128, TB, L, 65], DT_PV, name="vsb")

    def load_first_chunk():
        # per-k (wqk, xt) pairs stream first so the qk groups ramp the PE
        # with the DMA; wv only gates the v group which runs third
        xt_sb[0] = []
        for k in range(KT):
            nc.sync.dma_start(wqk_sb[k][:], wqk[k])
            xt = xpool.tile([128, CH], DT_X, name=f"xt0_{k}", tag="xt")
            nc.sync.dma_start(xt[:], xt4[0, k])
            xt_sb[0].append(xt)
            if k == 0:
                cs_sb[0] = cspool.tile([128, CH], DT_K, name="cs0", tag="cs")
                sn_sb[0] = cspool.tile([128, CH], DT_K, name="sn0", tag="sn")
                nc.sync.dma_start(cs_sb[0][:], cs4[0])
                nc.sync.dma_start(sn_sb[0][:], sn4[0])
            if k == 1:
                nc.sync.dma_start(stair_sb[:], stair[:])
        for k in range(KT):
            nc.sync.dma_start(wv_sb[k][:], wv[k])
        # softmax-denominator ones column
        nc.vector.memset(v_sb[:, :, :, 64:65], 1.0)

    def load_consts_late():
        for p in range(NP):
            nc.sync.dma_start(wp_sb[p][:], wp[p])

    # per-chunk transient state
    xt_sb = {}
    q_rot = {}
    cs_sb = {}
    sn_sb = {}
    ot_sb = {}

    def load_chunk_inputs(c):
        us = []

        def mk_load(c):
            def f():
                cs_sb[c] = cspool.tile([128, CH], DT_K, name=f"cs{c}", tag="cs")
                sn_sb[c] = cspool.tile([128, CH], DT_K, name=f"sn{c}", tag="sn")
                nc.sync.dma_start(cs_sb[c][:], cs4[c])
                nc.sync.dma_start(sn_sb[c][:], sn4[c])
                xt_sb[c] = []
                for k in range(KT):
                    xt = xpool.tile([128, CH], DT_X, name=f"xt{c}_{k}", tag="xt")
                    nc.sync.dma_start(xt[:], xt4[c, k])
                    xt_sb[c].append(xt)

            return f

        us.append((mk_load(c), "pe"))
        return us

    def rope_tile(c, jt, ps, on_act):
        """RoPE for one [128, CH] q/k feature tile whose raw values sit in
        psum `ps`.  rot = raw*cos + swap(raw)*sin_signed.

        evac: psum -> bf16 q_sb (ACT engine when it is idle, else DVE);
        4 small swap copies (bf16, 4x mode); two TT mults + one TT add.
        """
        sn = sn_sb[c]
        q_sb = rtmp.tile([128, CH], DT_K, name=f"qsb{c}_{jt}", tag="qsb")
        if on_act:
            nc.scalar.copy(q_sb[:], ps[:])
        else:
            nc.vector.tensor_copy(q_sb[:], ps[:])
        qsw = rtmp.tile([128, CH], DT_K, name=f"qsw{c}_{jt}", tag="qsw")
        for blk in range(2):
            b0 = blk * 64
            nc.vector.tensor_copy(qsw[b0 : b0 + 32, :], q_sb[b0 + 32 : b0 + 64, :])
            nc.vector.tensor_copy(qsw[b0 + 32 : b0 + 64, :], q_sb[b0 : b0 + 32, :])
        qtmp = rtmp.tile([128, CH], DT_K, name=f"qtmp{c}_{jt}", tag="qtmp")
        nc.vector.tensor_tensor(
            out=qtmp[:], in0=qsw[:], in1=sn[:], op=mybir.AluOpType.mult
        )
        qraw = rtmp.tile([128, CH], DT_K, name=f"qraw{c}_{jt}", tag="qraw")
        nc.vector.tensor_tensor(
            out=qraw[:], in0=q_sb[:], in1=cs_sb[c][:], op=mybir.AluOpType.mult
        )
        if jt < NP:  # q tile
            dst = qrpool.tile([128, CH], DT_K, name=f"qrot{c}_{jt}", tag="qr")
            q_rot[(c, jt)] = dst
            nc.vector.tensor_tensor(
                out=dst[:], in0=qraw[:], in1=qtmp[:], op=mybir.AluOpType.add
            )
        else:  # k tile
            p = jt - NP
            nc.vector.tensor_tensor(
                out=k_rot[p][:, c * CH : (c + 1) * CH],
                in0=qraw[:],
                in1=qtmp[:],
                op=mybir.AluOpType.add,
            )

    def v_evac(c, tbl, ps, on_act):
        tb = c * 4 + tbl
        dst = v_sb[:, tb, :, 0:64]
        src = ps[:].rearrange("p (h d) -> p h d", h=L)
        if on_act:
            nc.scalar.copy(dst, src)
        else:
            nc.vector.tensor_copy(dst, src)

    def stage_a0_units():
        """Chunk 0 stage A with k-outer matmul groups of 4 units so the PE
        ramps while the first DMAs stream in; psum evacuations go to the
        idle ACT engine."""
        units = []
        groups = [[0, 4, 1, 5], [2, 6, 3, 7], [8, 9, 10, 11]]  # 8..11 = v tbl 0..3

        def mk_group(gi, group):
            def f():
                pss = {}
                for i, u in enumerate(group):
                    pool = pmisc if i < 2 else pss_pool
                    shape = [128, CH] if i < 2 else [128, 2 * CH]
                    tag = "pA" if i < 2 else "pss"
                    pss[u] = pool.tile(shape, F32, name=f"psA0g{gi}_{u}", tag=tag)
                for k in range(KT):
                    for u in group:
                        ps = pss[u][:, 0:CH]
                        if u < 8:  # q/k feature tile
                            nc.tensor.matmul(
                                ps,
                                wqk_sb[k][:, u * 128 : (u + 1) * 128],
                                xt_sb[0][k][:],
                                start=(k == 0),
                                stop=(k == KT - 1),
                            )
                        else:  # v token block
                            tbl = u - 8
                            nc.tensor.matmul(
                                ps,
                                xt_sb[0][k][:, tbl * 128 : (tbl + 1) * 128],
                                wv_sb[k][:],
                                start=(k == 0),
                                stop=(k == KT - 1),
                            )
                for u in group:
                    if u < 8:
                        rope_tile(0, u, pss[u][:, 0:CH], on_act=True)
                    else:
                        v_evac(0, u - 8, pss[u][:, 0:CH], on_act=True)

            return f

        for gi, g in enumerate(groups):
            units.append(mk_group(gi, g))
        return units

    def stage_a_units(c):
        """8 q/k feature tiles + 4 v t-blocks for chunk c, each split into
        two fill sub-units (4 contraction matmuls each) so interleaved
        fill blobs stay small."""
        units = []
        ps_live = {}

        def mk_qk(c, jt, half):
            def f():
                if half == 0:
                    ps_live[jt] = pmisc.tile(
                        [128, CH], F32, name=f"psA{c}_{jt}", tag="pA"
                    )
                ps = ps_live[jt]
                for k in range(half * 4, half * 4 + 4):
                    nc.tensor.matmul(
                        ps[:],
                        wqk_sb[k][:, jt * 128 : (jt + 1) * 128],
                        xt_sb[c][k][:],
                        start=(k == 0),
                        stop=(k == KT - 1),
                    )
                if half == 1:
                    rope_tile(c, jt, ps_live.pop(jt), on_act=False)

            return f

        def mk_v(c, tbl, half):
            def f():
                if half == 0:
                    ps_live[8 + tbl] = pmisc.tile(
                        [128, CH], F32, name=f"psV{c}_{tbl}", tag="pA"
                    )
                ps = ps_live[8 + tbl]
                for k in range(half * 4, half * 4 + 4):
                    nc.tensor.matmul(
                        ps[:],
                        xt_sb[c][k][:, tbl * 128 : (tbl + 1) * 128],
                        wv_sb[k][:],
                        start=(k == 0),
                        stop=(k == KT - 1),
                    )
                if half == 1:
                    v_evac(c, tbl, ps_live.pop(8 + tbl), on_act=False)

            return f

        for jt in range(2 * NP):
            units.append((mk_qk(c, jt, 0), "pe"))
            units.append((mk_qk(c, jt, 1), "dve"))
        for tbl in range(4):
            units.append((mk_v(c, tbl, 0), "pe"))
            units.append((mk_v(c, tbl, 1), "dve"))
        return units

    def stage_b_units(c):
        """per chunk c: 4 head-pairs x (4c+4) key blocks, software-pipelined.

        Sub-unit (p, jb).qk: one [128, 2*CH] psum tile = both heads' S_T
        scores for key block jb (head a cols 0:CH, head b cols CH:2CH).
        The two K=64 QK matmuls are adjacent and land in different PE row
        groups (partitions 0:64 vs 64:128) so they stream concurrently.
        One exp on ACT covers both heads; diagonal blocks shrink to
        columns >= io and get a staircase mask on 128 cols.

        Sub-unit (p, jb).pv consumes the exp'd tile.  Emission runs the
        pv of block jb AFTER the qk of block jb+1 so the PE has covering
        work while the ACT exp is in flight.
        """
        units = []
        nblk = 4 * c + 4

        def mk_qk(c, p, jb):
            def f():
                qt = q_rot[(c, p)]
                kt_ = k_rot[p]
                io = (jb - 4 * c) * 128 if jb >= 4 * c else 0
                pss = pss_pool.tile(
                    [128, 2 * CH], F32, name=f"pss{c}_{p}_{jb}", tag="pss"
                )
                pss_unit[(p, jb)] = pss
                for idx, rb in ((0, 0), (1, 64)):
                    nc.tensor.matmul(
                        pss[:, idx * CH + io : (idx + 1) * CH],
                        kt_[rb : rb + 64, jb * 128 : (jb + 1) * 128],
                        qt[rb : rb + 64, io:],
                        start=True,
                        stop=True,
                    )
                pt = ptpool.tile([128, 2 * CH], DT_PV, name=f"pt{c}_{p}_{jb}", tag="pt")
                pt_unit[(p, jb)] = pt
                if io == 0:
                    nc.scalar.activation(
                        pt[:],
                        pss[:],
                        mybir.ActivationFunctionType.Exp,
                        scale=float(SCALE),
                    )
                else:
                    src = pss[:].rearrange("q (h t) -> q h t", h=2)[:, :, io:]
                    dst = pt[:].rearrange("q (h t) -> q h t", h=2)[:, :, io:]
                    nc.scalar.activation(
                        dst, src, mybir.ActivationFunctionType.Exp, scale=float(SCALE)
                    )
                if jb >= 4 * c:  # diagonal block -> staircase mask on 128 cols
                    pv = pt[:].rearrange("q (h t) -> q h t", h=2)[:, :, io : io + 128]
                    nc.vector.tensor_tensor(
                        out=pv, in0=pv, in1=stair_sb[:], op=mybir.AluOpType.mult
                    )

            return f

        def mk_pv(c, p, jb):
            def f():
                pt = pt_unit.pop((p, jb))
                io = (jb - 4 * c) * 128 if jb >= 4 * c else 0
                for idx, h in ((0, 2 * p), (1, 2 * p + 1)):
                    if jb == 0:
                        pso_unit[(c, h)] = pso_pool.tile(
                            [65, CH], F32, name=f"pso{c}_{h}", tag="pso"
                        )
                    pso = pso_unit[(c, h)]
                    nc.tensor.matmul(
                        pso[:, io:],
                        v_sb[:, jb, h, 0:65],
                        pt[:, idx * CH + io : (idx + 1) * CH],
                        start=(jb == 0),
                        stop=(jb == nblk - 1),
                    )

            return f

        def mk_evac(c, p, idx):
            """One copy evacuates o_raw + denominator and frees the pso
            bank immediately; the normalize runs later off-critical-path."""

            def f():
                h = 2 * p + idx
                pso = pso_unit[(c, h)]
                lsb = lsb_sb[(c, h)] = lpool.tile(
                    [1, CH], F32, name=f"lsb{c}_{h}", tag="lsb"
                )
                nc.vector.tensor_copy(lsb[:], pso[64:65, :])
                oraw = oraw_sb[(c, h)] = lpool.tile(
                    [64, CH], F32, name=f"oraw{c}_{h}", tag="oraw"
                )
                nc.vector.tensor_copy(oraw[:], pso[0:64, :])

            return f

        def mk_finish(c, p, idx):
            def f():
                if idx == 0:
                    ot_sb[(c, p)] = otpool.tile(
                        [128, CH], DT_O, name=f"ot{c}_{p}", tag="ot"
                    )
                ot = ot_sb[(c, p)]
                h = 2 * p + idx
                oraw = oraw_sb.pop((c, h))
                lsb = lsb_sb.pop((c, h))
                linv = lpool.tile([1, CH], F32, name=f"linv{c}_{h}", tag="linv")
                nc.vector.reciprocal_approx_fast(linv[:], lsb[:])
                lb = lpool.tile([64, CH], F32, name=f"lb{c}_{h}", tag="lb")
                nc.gpsimd.partition_broadcast(lb[:], linv[:])
                nc.vector.tensor_tensor(
                    out=ot[idx * 64 : (idx + 1) * 64, :],
                    in0=oraw[0:64, :],
                    in1=lb[:],
                    op=mybir.AluOpType.mult,
                )

            return f

        pso_unit = {}
        pss_unit = {}
        pt_unit = {}
        oraw_sb = {}
        lsb_sb = {}
        # software pipeline across the whole chunk (pairs run back-to-back):
        # qk(i+1) is emitted before pv(i) so exp latency is covered.  Units
        # are (fn, fill_weight); norms get extra fill to cover the
        # reciprocal/broadcast chain that gates the next pair's PV.
        from collections import deque

        seq = [(p, jb) for p in range(NP) for jb in range(nblk)]
        DEPTH = 2  # qk runs this many units ahead of pv
        norm_q = deque()

        def push_norm():
            if norm_q:
                units.append((norm_q.popleft(), 2, True))

        def push_pv(j):
            units.append((mk_pv(c, *seq[j]), 1, False))
            if seq[j][1] == nblk - 1:  # pair done: evac both heads right
                p = seq[j][0]  # away, defer the normalize a few units
                units.append((mk_evac(c, p, 0), 2, True))
                units.append((mk_evac(c, p, 1), 1, False))
                norm_q.append(mk_finish(c, p, 0))
                norm_q.append(mk_finish(c, p, 1))

        for i, s in enumerate(seq):
            w = 3 if i < 2 else 1  # front-load fill to cover the previous
            units.append((mk_qk(c, *s), w, False))  # chunk's trailing evacs
            push_norm()
            if i - DEPTH >= 0:
                push_pv(i - DEPTH)
            push_norm()
        for j in range(len(seq) - DEPTH, len(seq)):
            push_pv(j)
            push_norm()
        while norm_q:
            push_norm()
        return units

    def proj_units(c, evac_act=False):
        """Both oc halves per unit: consecutive matmuls share the ot lhsT
        so only one weight load is paid per contraction step."""
        units = []

        def mk_proj(c, tbl):
            def f():
                pss = [
                    pmisc.tile([128, CH], F32, name=f"psY{c}_{tbl}_{oc}", tag="pA")
                    for oc in range(2)
                ]
                for p in range(NP):
                    for oc in range(2):
                        nc.tensor.matmul(
                            pss[oc][:],
                            ot_sb[(c, p)][:, tbl * 128 : (tbl + 1) * 128],
                            wp_sb[p][:, oc * CH : (oc + 1) * CH],
                            start=(p == 0),
                            stop=(p == NP - 1),
                        )
                for oc in range(2):
                    ye = yepool.tile(
                        [128, CH], DT_O, name=f"ye{c}_{tbl}_{oc}", tag="ye"
                    )
                    if evac_act:
                        nc.scalar.copy(ye[:], pss[oc][:])
                    else:
                        nc.vector.tensor_copy(ye[:], pss[oc][:])
                    nc.sync.dma_start(
                        y[c * CH + tbl * 128 : c * CH + (tbl + 1) * 128,
                          oc * CH : (oc + 1) * CH],
                        ye[:],
                    )

            return f

        for tbl in range(4):
            units.append((mk_proj(c, tbl), "dve"))
        return units

    def emit_interleaved(primary, secondary, reserve=0.10):
        """Emit weighted primary units (fn, w, pe_only) with secondary
        units (fn, kind) spread between them proportionally to weight.
        At pe_only slots (norm boundaries) prefer pure-matmul fill so no
        extra DVE work queues ahead of the norm chain; hold back a
        fraction of secondary for the phase tail."""
        if not primary:
            for u, _ in secondary:
                u()
            return
        nres = int(len(secondary) * reserve)
        spread = spread_l = secondary[: len(secondary) - nres]
        ns = len(spread)
        emitted = [False] * ns
        ndone = 0

        def pull(pe_only):
            nonlocal ndone
            idx = None
            skipped = 0
            for j in range(ns):
                if emitted[j]:
                    continue
                if not pe_only or spread[j][1] == "pe":
                    idx = j
                    break
                skipped += 1
                if skipped > 3:
                    break
            if idx is None:
                for j in range(ns):
                    if not emitted[j]:
                        idx = j
                        break
            if idx is None:
                return False
            emitted[idx] = True
            spread[idx][0]()
            ndone += 1
            return True

        wtot = sum(w for _, w, _ in primary)
        cum = 0
        for u, w, pe_only in primary:
            u()
            cum += w
            want = cum * ns // wtot
            while ndone < want:
                if not pull(pe_only):
                    break
        for j in range(ns):
            if not emitted[j]:
                spread[j][0]()
        for u, _ in secondary[len(secondary) - nres :]:
            u()

    # ---- emission ----
    load_first_chunk()
    for u in stage_a0_units():
        u()
    load_consts_late()
    for c in range(NCH):
        fill = []
        if c + 1 < NCH:
            fill += load_chunk_inputs(c + 1)
            fill += stage_a_units(c + 1)
        if c == NCH - 1:
            # all deferred projections fill the ACT-bound final chunk
            for cc in range(NCH - 1):
                fill += proj_units(cc)
        emit_interleaved(stage_b_units(c), fill)
    for u, _ in proj_units(NCH - 1, evac_act=True):
        u()


def build_nc():
    nc = bacc.Bacc("TRN2", target_bir_lowering=False, debug=False)
    xt4 = nc.declare_dram_parameter("xt4", [NCH, KT, 128, CH], DT_X, isOutput=False)
    wqk = nc.declare_dram_parameter("wqk", [KT, 128, 1024], DT_X, isOutput=False)
    wv = nc.declare_dram_parameter("wv", [KT, 128, 512], DT_X, isOutput=False)
    wp = nc.declare_dram_parameter("wp", [NP, 128, C], DT_O, isOutput=False)
    cs4 = nc.declare_dram_parameter("cs4", [NCH, 128, CH], DT_K, isOutput=False)
    sn4 = nc.declare_dram_parameter("sn4", [NCH, 128, CH], DT_K, isOutput=False)
    stair = nc.declare_dram_parameter("stair", [128, 2, 128], DT_PV, isOutput=False)
    yout = nc.declare_dram_parameter("y", [T, C], DT_O, isOutput=True)

    with tile.TileContext(nc) as tc:
        with ExitStack() as ctx:
            attn_body(
                ctx, tc, (yout[:],),
                (xt4[:], wqk[:], wv[:], wp[:], cs4[:], sn4[:], stair[:]),
            )
    nc.compile()
    return nc


# ---------------- host side ----------------


def _rope_tables_np():
    inv_freq = 1.0 / (ROPE_BASE ** (np.arange(0, D, 2, dtype=np.float64) / D))
    t = np.arange(T, dtype=np.float64)
    freqs = np.outer(t, inv_freq)  # [T, 32]
    emb = np.concatenate([freqs, freqs], axis=-1)  # [T, 64]
    return np.cos(emb), np.sin(emb)  # [T, 64] each


def _host_tables():
    cos, sin = _rope_tables_np()  # [T, 64]
    d_of_r = np.arange(128) % 64
    cs = cos[:, d_of_r].T.astype(np.float32)  # [128, T]
    sn_abs = sin[:, d_of_r].T
    sign = np.where((d_of_r % 64) < 32, -1.0, 1.0)[:, None]
    sn = (sn_abs * sign).astype(np.float32)  # [128, T]
    np_k = _np_dt(DT_K)
    cs4 = np.ascontiguousarray(cs.reshape(128, NCH, CH).transpose(1, 0, 2)).astype(np_k)
    sn4 = np.ascontiguousarray(sn.reshape(128, NCH, CH).transpose(1, 0, 2)).astype(np_k)

    # universal diagonal staircase [128 key rows, 2 (head copies), 128 cols]:
    # valid iff key-within-block <= col-within-staircase
    jj = np.arange(128)[:, None]
    xx = np.arange(128)[None, :]
    st = (jj <= xx).astype(np.float64)
    stair = np.stack([st, st], axis=1)  # [128, 2, 128]
    return cs4, sn4, stair


def make_core_inputs(x, Wqkv, Wproj, core):
    """Build the per-core input map (numpy arrays, device dtypes)."""
    b, g = core // 2, core % 2
    np_x = _np_dt(DT_X)
    np_pv = _np_dt(DT_PV)
    np_o = _np_dt(DT_O)

    xT = np.ascontiguousarray(x[b].T)  # [C, T]
    xt4 = np.ascontiguousarray(
        xT.reshape(KT, 128, NCH, CH).transpose(2, 0, 1, 3)
    ).astype(np_x)

    Wq = Wqkv[g * 512 : (g + 1) * 512]
    Wk = Wqkv[C + g * 512 : C + (g + 1) * 512]
    Wv = Wqkv[2 * C + g * 512 : 2 * C + (g + 1) * 512]
    wqkT = np.vstack([Wq, Wk]).T  # [C, 1024]
    wqk = np.ascontiguousarray(wqkT.reshape(KT, 128, 1024)).astype(np_x)
    wvT = Wv.T  # [C, 512]
    wv = np.ascontiguousarray(wvT.reshape(KT, 128, 512)).astype(np_x)
    wpT = Wproj[:, g * 512 : (g + 1) * 512].T  # [512, C]
    wp = np.ascontiguousarray(wpT.reshape(NP, 128, C)).astype(np_o)

    cs4, sn4, stair = _host_tables()
    return {
        "xt4": xt4,
        "wqk": wqk,
        "wv": wv,
        "wp": wp,
        "cs4": cs4,
        "sn4": sn4,
        "stair": stair.astype(np_pv),
    }


LAST_RESULTS = None
_NC_CACHE = None


def kernel(x, Wqkv, Wproj):
    global LAST_RESULTS, _NC_CACHE
    from concourse.bass_utils import run_bass_kernel_spmd

    x = np.asarray(x, dtype=np.float32)
    Wqkv = np.asarray(Wqkv, dtype=np.float32)
    Wproj = np.asarray(Wproj, dtype=np.float32)

    if _NC_CACHE is None:
        _NC_CACHE = build_nc()
    nc = _NC_CACHE
    in_maps = [make_core_inputs(x, Wqkv, Wproj, core) for core in range(NCORES)]
    res = run_bass_kernel_spmd(nc, in_maps, list(range(NCORES)))
    LAST_RESULTS = res

    out = np.empty((B, T, C), dtype=np.float32)
    for b in range(B):
        out[b] = res.results[2 * b]["y"].astype(np.float32) + res.results[
            2 * b + 1
        ]["y"].astype(np.float32)
    return out


# revision 37
# speedup vs baseline: 1.0303x; 1.0303x over previous
"""Causal self-attention (RoPE) Trainium2 Bass kernel.

Problem: B=4, T=2048, C=1024, H=16 heads, D=64, fp32 I/O.
Sharding: 8 cores = 4 (batch) x 2 (head-group TP). Each core computes
qkv/attention/proj for 1 batch and 8 heads, producing a partial
projection output; the host sums the two TP partials per batch.

Per-core pipeline (chunk-interleaved to overlap PE matmuls with the
ACT-engine exp of the softmax):
  stage A (per 512-token chunk c): qkv projection + RoPE -> q_rot(c)
    (transient), k_rot[:, c] (persistent), v(c) (persistent, with an
    appended ones column for the softmax denominator)
  stage B (per chunk c, head-pair p, key-block jb): one [128, 1024]
    psum tile holds both heads' S_T scores for key block jb; the two
    K=64 QK matmuls are adjacent with different row groups so they
    run concurrently in the PE array.  One exp on ACT covers both
    heads (2-range AP when the causal io-shrink applies), a [128,128]
    staircase mask on DVE for diagonal blocks, then per-head
    out_T = v_ext^T @ P_T accumulated over key blocks; row 64 of the
    accumulator is the softmax denominator l.  Normalize with
    reciprocal (read straight from PSUM) + gpsimd partition_broadcast.
  proj (per chunk): y[chunk] = o_T^T @ WprojT, partial over this
    core's 512 input features.
"""

import numpy as np
from contextlib import ExitStack

import concourse.bacc as bacc
import concourse.bass as bass
import concourse.mybir as mybir
import concourse.tile as tile

# ---------------- constants ----------------
B = 4
T = 2048
C = 1024
H = 16
D = 64
L = 8  # local heads per core
NCORES = 8
ROPE_BASE = 10000.0

CH = 512  # t-chunk size
NCH = T // CH  # 4 chunks
KT = C // 128  # 8 contraction tiles
NP = L // 2  # 4 head-pair tiles
SCALE = 1.0 / np.sqrt(D)

F32 = mybir.dt.float32
BF16 = mybir.dt.bfloat16

# matmul operand dtypes
DT_X = BF16  # x / Wqkv / Wv operands
DT_K = BF16  # q_rot / k_rot
DT_PV = BF16  # P tiles, v tiles, masks
DT_O = BF16  # o_T tiles / WprojT


def _np_dt(dt):
    return mybir.dt.np(dt)


# ---------------- device kernel ----------------


def attn_body(ctx: ExitStack, tc: tile.TileContext, outs, ins):
    """outs = (y [T, C] f32,); ins = (xt4, wqk, wv, wp, cs4, sn4, stair)."""
    nc = tc.nc
    (y,) = outs if isinstance(outs, (tuple, list)) else (outs,)
    xt4, wqk, wv, wp, cs4, sn4, stair = ins

    TB = T // 128  # 16 key blocks

    consts = ctx.enter_context(tc.tile_pool(name="consts", bufs=1))
    xpool = ctx.enter_context(tc.tile_pool(name="xpool", bufs=16))
    cspool = ctx.enter_context(tc.tile_pool(name="cspool", bufs=4))
    qrpool = ctx.enter_context(tc.tile_pool(name="qrpool", bufs=8))
    rtmp = ctx.enter_context(tc.tile_pool(name="rtmp", bufs=4))
    ptpool = ctx.enter_context(tc.tile_pool(name="ptpool", bufs=6))
    otpool = ctx.enter_context(tc.tile_pool(name="otpool", bufs=16))
    yepool = ctx.enter_context(tc.tile_pool(name="yepool", bufs=3))
    lpool = ctx.enter_context(tc.tile_pool(name="lpool", bufs=4))
    pmisc = ctx.enter_context(tc.tile_pool(name="pmisc", bufs=2, space="PSUM"))
    pss_pool = ctx.enter_context(tc.tile_pool(name="pss", bufs=2, space="PSUM"))
    pso_pool = ctx.enter_context(tc.tile_pool(name="pso", bufs=2, space="PSUM"))

    # persistent tiles
    wqk_sb = [consts.tile([128, 2 * 512], DT_X, name=f"wqk{k}") for k in range(KT)]
    wv_sb = [consts.tile([128, 512], DT_X, name=f"wv{k}") for k in range(KT)]
    wp_sb = [consts.tile([128, C], DT_O, name=f"wp{p}") for p in range(NP)]
    stair_sb = consts.tile([128, 2, 128], DT_PV, name="stair")
    k_rot = [consts.tile([128, T], DT_K, name=f"krot{p}") for p in range(NP)]
    v_sb = consts.tile([128, TB, L, 65], DT_PV, name="vsb")

    def load_first_chunk():
        # per-k (wqk, xt) pairs stream first so the qk groups ramp the PE
        # with the DMA; wv only gates the v group which runs third
        xt_sb[0] = []
        for k in range(KT):
            nc.sync.dma_start(wqk_sb[k][:], wqk[k])
            xt = xpool.tile([128, CH], DT_X, name=f"xt0_{k}", tag="xt")
            nc.sync.dma_start(xt[:], xt4[0, k])
            xt_sb[0].append(xt)
            if k == 0:
                cs_sb[0] = cspool.tile([128, CH], DT_K, name="cs0", tag="cs")
                sn_sb[0] = cspool.tile([128, CH], DT_K, name="sn0", tag="sn")
                nc.sync.dma_start(cs_sb[0][:], cs4[0])
                nc.sync.dma_start(sn_sb[0][:], sn4[0])
            if k == 1:
                nc.sync.dma_start(stair_sb[:], stair[:])
        for k in range(KT):
            nc.sync.dma_start(wv_sb[k][:], wv[k])
        # softmax-denominator ones column
        nc.vector.memset(v_sb[:, :, :, 64:65], 1.0)

    def load_consts_late():
        for p in range(NP):
            nc.sync.dma_start(wp_sb[p][:], wp[p])

    # per-chunk transient state
    xt_sb = {}
    q_rot = {}
    cs_sb = {}
    sn_sb = {}
    ot_sb = {}

    def load_chunk_inputs(c):
        us = []

        def mk_load(c):
            def f():
                cs_sb[c] = cspool.tile([128, CH], DT_K, name=f"cs{c}", tag="cs")
                sn_sb[c] = cspool.tile([128, CH], DT_K, name=f"sn{c}", tag="sn")
                nc.sync.dma_start(cs_sb[c][:], cs4[c])
                nc.sync.dma_start(sn_sb[c][:], sn4[c])
                xt_sb[c] = []
                for k in range(KT):
                    xt = xpool.tile([128, CH], DT_X, name=f"xt{c}_{k}", tag="xt")
                    nc.sync.dma_start(xt[:], xt4[c, k])
                    xt_sb[c].append(xt)

            return f

        us.append((mk_load(c), "pe"))
        return us

    def rope_tile(c, jt, ps, on_act):
        """RoPE for one [128, CH] q/k feature tile whose raw values sit in
        psum `ps`.  rot = raw*cos + swap(raw)*sin_signed.

        evac: psum -> bf16 q_sb (ACT engine when it is idle, else DVE);
        4 small swap copies (bf16, 4x mode); two TT mults + one TT add.
        """
        sn = sn_sb[c]
        q_sb = rtmp.tile([128, CH], DT_K, name=f"qsb{c}_{jt}", tag="qsb")
        if on_act:
            nc.scalar.copy(q_sb[:], ps[:])
        else:
            nc.vector.tensor_copy(q_sb[:], ps[:])
        qsw = rtmp.tile([128, CH], DT_K, name=f"qsw{c}_{jt}", tag="qsw")
        for blk in range(2):
            b0 = blk * 64
            nc.vector.tensor_copy(qsw[b0 : b0 + 32, :], q_sb[b0 + 32 : b0 + 64, :])
            nc.vector.tensor_copy(qsw[b0 + 32 : b0 + 64, :], q_sb[b0 : b0 + 32, :])
        qtmp = rtmp.tile([128, CH], DT_K, name=f"qtmp{c}_{jt}", tag="qtmp")
        nc.vector.tensor_tensor(
            out=qtmp[:], in0=qsw[:], in1=sn[:], op=mybir.AluOpType.mult
        )
        qraw = rtmp.tile([128, CH], DT_K, name=f"qraw{c}_{jt}", tag="qraw")
        nc.vector.tensor_tensor(
            out=qraw[:], in0=q_sb[:], in1=cs_sb[c][:], op=mybir.AluOpType.mult
        )
        if jt < NP:  # q tile
            dst = qrpool.tile([128, CH], DT_K, name=f"qrot{c}_{jt}", tag="qr")
            q_rot[(c, jt)] = dst
            nc.vector.tensor_tensor(
                out=dst[:], in0=qraw[:], in1=qtmp[:], op=mybir.AluOpType.add
            )
        else:  # k tile
            p = jt - NP
            nc.vector.tensor_tensor(
                out=k_rot[p][:, c * CH : (c + 1) * CH],
                in0=qraw[:],
                in1=qtmp[:],
                op=mybir.AluOpType.add,
            )

    def v_evac(c, tbl, ps, on_act):
        tb = c * 4 + tbl
        dst = v_sb[:, tb, :, 0:64]
        src = ps[:].rearrange("p (h d) -> p h d", h=L)
        if on_act:
            nc.scalar.copy(dst, src)
        else:
            nc.vector.tensor_copy(dst, src)

    def stage_a0_units():
        """Chunk 0 stage A with k-outer matmul groups of 4 units so the PE
        ramps while the first DMAs stream in; psum evacuations go to the
        idle ACT engine."""
        units = []
        groups = [[0, 4, 1, 5], [2, 6, 3, 7], [8, 9, 10, 11]]  # 8..11 = v tbl 0..3

        def mk_group(gi, group):
            def f():
                pss = {}
                for i, u in enumerate(group):
                    pool = pmisc if i < 2 else pss_pool
                    shape = [128, CH] if i < 2 else [128, 2 * CH]
                    tag = "pA" if i < 2 else "pss"
                    pss[u] = pool.tile(shape, F32, name=f"psA0g{gi}_{u}", tag=tag)
                for k in range(KT):
                    for u in group:
                        ps = pss[u][:, 0:CH]
                        if u < 8:  # q/k feature tile
                            nc.tensor.matmul(
                                ps,
                                wqk_sb[k][:, u * 128 : (u + 1) * 128],
                                xt_sb[0][k][:],
                                start=(k == 0),
                                stop=(k == KT - 1),
                            )
                        else:  # v token block
                            tbl = u - 8
                            nc.tensor.matmul(
                                ps,
                                xt_sb[0][k][:, tbl * 128 : (tbl + 1) * 128],
                                wv_sb[k][:],
                                start=(k == 0),
                                stop=(k == KT - 1),
                            )
                for u in group:
                    if u < 8:
                        rope_tile(0, u, pss[u][:, 0:CH], on_act=True)
                    else:
                        v_evac(0, u - 8, pss[u][:, 0:CH], on_act=True)

            return f

        for gi, g in enumerate(groups):
            units.append(mk_group(gi, g))
        return units

    def stage_a_units(c):
        """8 q/k feature tiles + 4 v t-blocks for chunk c, each split into
        two fill sub-units (4 contraction matmuls each) so interleaved
        fill blobs stay small."""
        units = []
        ps_live = {}

        def mk_qk(c, jt, half):
            def f():
                if half == 0:
                    ps_live[jt] = pmisc.tile(
                        [128, CH], F32, name=f"psA{c}_{jt}", tag="pA"
                    )
                ps = ps_live[jt]
                for k in range(half * 4, half * 4 + 4):
                    nc.tensor.matmul(
                        ps[:],
                        wqk_sb[k][:, jt * 128 : (jt + 1) * 128],
                        xt_sb[c][k][:],
                        start=(k == 0),
                        stop=(k == KT - 1),
                    )
                if half == 1:
                    rope_tile(c, jt, ps_live.pop(jt), on_act=False)

            return f

        def mk_v(c, tbl, half):
            def f():
                if half == 0:
                    ps_live[8 + tbl] = pmisc.tile(
                        [128, CH], F32, name=f"psV{c}_{tbl}", tag="pA"
                    )
                ps = ps_live[8 + tbl]
                for k in range(half * 4, half * 4 + 4):
                    nc.tensor.matmul(
                        ps[:],
                        xt_sb[c][k][:, tbl * 128 : (tbl + 1) * 128],
                        wv_sb[k][:],
                        start=(k == 0),
                        stop=(k == KT - 1),
                    )
                if half == 1:
                    v_evac(c, tbl, ps_live.pop(8 + tbl), on_act=False)

            return f

        for jt in range(2 * NP):
            units.append((mk_qk(c, jt, 0), "pe"))
            units.append((mk_qk(c, jt, 1), "dve"))
        for tbl in range(4):
            units.append((mk_v(c, tbl, 0), "pe"))
            units.append((mk_v(c, tbl, 1), "dve"))
        return units

    def stage_b_units(c):
        """per chunk c: 4 head-pairs x (4c+4) key blocks, software-pipelined.

        Sub-unit (p, jb).qk: one [128, 2*CH] psum tile = both heads' S_T
        scores for key block jb (head a cols 0:CH, head b cols CH:2CH).
        The two K=64 QK matmuls are adjacent and land in different PE row
        groups (partitions 0:64 vs 64:128) so they stream concurrently.
        One exp on ACT covers both heads; diagonal blocks shrink to
        columns >= io and get a staircase mask on 128 cols.

        Sub-unit (p, jb).pv consumes the exp'd tile.  Emission runs the
        pv of block jb AFTER the qk of block jb+1 so the PE has covering
        work while the ACT exp is in flight.
        """
        units = []
        nblk = 4 * c + 4

        def mk_qk(c, p, jb):
            def f():
                qt = q_rot[(c, p)]
                kt_ = k_rot[p]
                io = (jb - 4 * c) * 128 if jb >= 4 * c else 0
                pss = pss_pool.tile(
                    [128, 2 * CH], F32, name=f"pss{c}_{p}_{jb}", tag="pss"
                )
                pss_unit[(p, jb)] = pss
                for idx, rb in ((0, 0), (1, 64)):
                    nc.tensor.matmul(
                        pss[:, idx * CH + io : (idx + 1) * CH],
                        kt_[rb : rb + 64, jb * 128 : (jb + 1) * 128],
                        qt[rb : rb + 64, io:],
                        start=True,
                        stop=True,
                    )
                pt = ptpool.tile([128, 2 * CH], DT_PV, name=f"pt{c}_{p}_{jb}", tag="pt")
                pt_unit[(p, jb)] = pt
                if io == 0:
                    nc.scalar.activation(
                        pt[:],
                        pss[:],
                        mybir.ActivationFunctionType.Exp,
                        scale=float(SCALE),
                    )
                else:
                    src = pss[:].rearrange("q (h t) -> q h t", h=2)[:, :, io:]
                    dst = pt[:].rearrange("q (h t) -> q h t", h=2)[:, :, io:]
                    nc.scalar.activation(
                        dst, src, mybir.ActivationFunctionType.Exp, scale=float(SCALE)
                    )
                if jb >= 4 * c:  # diagonal block -> staircase mask on 128 cols
                    # keep pt[p, h, x] where x >= p, else 0 — on the idle
                    # gpsimd engine so no mask work queues on the DVE
                    pv = pt[:].rearrange("q (h t) -> q h t", h=2)[:, :, io : io + 128]
                    nc.gpsimd.affine_select(
                        out=pv,
                        in_=pv,
                        compare_op=mybir.AluOpType.is_ge,
                        fill=0.0,
                        base=0,
                        pattern=[[0, 2], [1, 128]],
                        channel_multiplier=-1,
                    )

            return f

        def mk_pv(c, p, jb):
            def f():
                pt = pt_unit.pop((p, jb))
                io = (jb - 4 * c) * 128 if jb >= 4 * c else 0
                for idx, h in ((0, 2 * p), (1, 2 * p + 1)):
                    if jb == 0:
                        pso_unit[(c, h)] = pso_pool.tile(
                            [65, CH], F32, name=f"pso{c}_{h}", tag="pso"
                        )
                    pso = pso_unit[(c, h)]
                    nc.tensor.matmul(
                        pso[:, io:],
                        v_sb[:, jb, h, 0:65],
                        pt[:, idx * CH + io : (idx + 1) * CH],
                        start=(jb == 0),
                        stop=(jb == nblk - 1),
                    )

            return f

        def mk_norm(c, p, idx):
            def f():
                if idx == 0:
                    ot_sb[(c, p)] = otpool.tile(
                        [128, CH], DT_O, name=f"ot{c}_{p}", tag="ot"
                    )
                ot = ot_sb[(c, p)]
                h = 2 * p + idx
                pso = pso_unit[(c, h)]
                lsb = lpool.tile([1, CH], F32, name=f"lsb{c}_{h}", tag="lsb")
                nc.vector.tensor_copy(lsb[:], pso[64:65, :])
                linv = lpool.tile([1, CH], F32, name=f"linv{c}_{h}", tag="linv")
                nc.vector.reciprocal_approx_fast(linv[:], lsb[:])
                lb = lpool.tile([64, CH], F32, name=f"lb{c}_{h}", tag="lb")
                nc.gpsimd.partition_broadcast(lb[:], linv[:])
                nc.vector.tensor_tensor(
                    out=ot[idx * 64 : (idx + 1) * 64, :],
                    in0=pso[0:64, :],
                    in1=lb[:],
                    op=mybir.AluOpType.mult,
                )

            return f

        pso_unit = {}
        pss_unit = {}
        pt_unit = {}
        oraw_sb = {}
        lsb_sb = {}
        # software pipeline across the whole chunk (pairs run back-to-back):
        # qk(i+1) is emitted before pv(i) so exp latency is covered.  Units
        # are (fn, fill_weight); norms get extra fill to cover the
        # reciprocal/broadcast chain that gates the next pair's PV.
        from collections import deque

        seq = [(p, jb) for p in range(NP) for jb in range(nblk)]
        DEPTH = 2  # qk runs this many units ahead of pv
        norm_q = deque()

        def push_norm():
            if norm_q:
                units.append((norm_q.popleft(), 4, True))

        def push_pv(j):
            units.append((mk_pv(c, *seq[j]), 1, False))
            if seq[j][1] == nblk - 1:  # pair done -> queue both heads' norms
                norm_q.append(mk_norm(c, seq[j][0], 0))
                norm_q.append(mk_norm(c, seq[j][0], 1))

        for i, s in enumerate(seq):
            w = 3 if i < 2 else 1  # front-load fill to cover the previous
            units.append((mk_qk(c, *s), w, False))  # chunk's trailing evacs
            push_norm()
            if i - DEPTH >= 0:
                push_pv(i - DEPTH)
            push_norm()
        for j in range(len(seq) - DEPTH, len(seq)):
            push_pv(j)
            push_norm()
        while norm_q:
            push_norm()
        return units

    def proj_units(c, evac_act=False):
        """Both oc halves per unit: consecutive matmuls share the ot lhsT
        so only one weight load is paid per contraction step."""
        units = []

        def mk_proj(c, tbl):
            def f():
                pss = [
                    pmisc.tile([128, CH], F32, name=f"psY{c}_{tbl}_{oc}", tag="pA")
                    for oc in range(2)
                ]
                for p in range(NP):
                    for oc in range(2):
                        nc.tensor.matmul(
                            pss[oc][:],
                            ot_sb[(c, p)][:, tbl * 128 : (tbl + 1) * 128],
                            wp_sb[p][:, oc * CH : (oc + 1) * CH],
                            start=(p == 0),
                            stop=(p == NP - 1),
                        )
                for oc in range(2):
                    ye = yepool.tile(
                        [128, CH], DT_O, name=f"ye{c}_{tbl}_{oc}", tag="ye"
                    )
                    if evac_act:
                        nc.scalar.copy(ye[:], pss[oc][:])
                    else:
                        nc.vector.tensor_copy(ye[:], pss[oc][:])
                    nc.sync.dma_start(
                        y[c * CH + tbl * 128 : c * CH + (tbl + 1) * 128,
                          oc * CH : (oc + 1) * CH],
                        ye[:],
                    )

            return f

        for tbl in range(4):
            units.append((mk_proj(c, tbl), "dve"))
        return units

    def emit_interleaved(primary, secondary, reserve=0.10):
        """Emit weighted primary units (fn, w, pe_only) with secondary
        units (fn, kind) spread between them proportionally to weight.
        At pe_only slots (norm boundaries) prefer pure-matmul fill so no
        extra DVE work queues ahead of the norm chain; hold back a
        fraction of secondary for the phase tail."""
        if not primary:
            for u, _ in secondary:
                u()
            return
        nres = int(len(secondary) * reserve)
        spread = spread_l = secondary[: len(secondary) - nres]
        ns = len(spread)
        emitted = [False] * ns
        ndone = 0

        def pull(pe_only):
            nonlocal ndone
            idx = None
            skipped = 0
            for j in range(ns):
                if emitted[j]:
                    continue
                if not pe_only or spread[j][1] == "pe":
                    idx = j
                    break
                skipped += 1
                if skipped > 3:
                    break
            if idx is None:
                for j in range(ns):
                    if not emitted[j]:
                        idx = j
                        break
            if idx is None:
                return False
            emitted[idx] = True
            spread[idx][0]()
            ndone += 1
            return True

        wtot = sum(w for _, w, _ in primary)
        cum = 0
        for u, w, pe_only in primary:
            u()
            cum += w
            want = cum * ns // wtot
            while ndone < want:
                if not pull(pe_only):
                    break
        for j in range(ns):
            if not emitted[j]:
                spread[j][0]()
        for u, _ in secondary[len(secondary) - nres :]:
            u()

    # ---- emission ----
    load_first_chunk()
    for u in stage_a0_units():
        u()
    load_consts_late()
    for c in range(NCH):
        fill = []
        if c + 1 < NCH:
            fill += load_chunk_inputs(c + 1)
            fill += stage_a_units(c + 1)
        if c == NCH - 1:
            # all deferred projections fill the ACT-bound final chunk
            for cc in range(NCH - 1):
                fill += proj_units(cc)
        emit_interleaved(stage_b_units(c), fill)
    for u, _ in proj_units(NCH - 1, evac_act=True):
        u()


def build_nc():
    nc = bacc.Bacc("TRN2", target_bir_lowering=False, debug=False)
    xt4 = nc.declare_dram_parameter("xt4", [NCH, KT, 128, CH], DT_X, isOutput=False)
    wqk = nc.declare_dram_parameter("wqk", [KT, 128, 1024], DT_X, isOutput=False)
    wv = nc.declare_dram_parameter("wv", [KT, 128, 512], DT_X, isOutput=False)
    wp = nc.declare_dram_parameter("wp", [NP, 128, C], DT_O, isOutput=False)
    cs4 = nc.declare_dram_parameter("cs4", [NCH, 128, CH], DT_K, isOutput=False)
    sn4 = nc.declare_dram_parameter("sn4", [NCH, 128, CH], DT_K, isOutput=False)
    stair = nc.declare_dram_parameter("stair", [128, 2, 128], DT_PV, isOutput=False)
    yout = nc.declare_dram_parameter("y", [T, C], DT_O, isOutput=True)

    with tile.TileContext(nc) as tc:
        with ExitStack() as ctx:
            attn_body(
                ctx, tc, (yout[:],),
                (xt4[:], wqk[:], wv[:], wp[:], cs4[:], sn4[:], stair[:]),
            )
    nc.compile()
    return nc


# ---------------- host side ----------------


def _rope_tables_np():
    inv_freq = 1.0 / (ROPE_BASE ** (np.arange(0, D, 2, dtype=np.float64) / D))
    t = np.arange(T, dtype=np.float64)
    freqs = np.outer(t, inv_freq)  # [T, 32]
    emb = np.concatenate([freqs, freqs], axis=-1)  # [T, 64]
    return np.cos(emb), np.sin(emb)  # [T, 64] each


def _host_tables():
    cos, sin = _rope_tables_np()  # [T, 64]
    d_of_r = np.arange(128) % 64
    cs = cos[:, d_of_r].T.astype(np.float32)  # [128, T]
    sn_abs = sin[:, d_of_r].T
    sign = np.where((d_of_r % 64) < 32, -1.0, 1.0)[:, None]
    sn = (sn_abs * sign).astype(np.float32)  # [128, T]
    np_k = _np_dt(DT_K)
    cs4 = np.ascontiguousarray(cs.reshape(128, NCH, CH).transpose(1, 0, 2)).astype(np_k)
    sn4 = np.ascontiguousarray(sn.reshape(128, NCH, CH).transpose(1, 0, 2)).astype(np_k)

    # universal diagonal staircase [128 key rows, 2 (head copies), 128 cols]:
    # valid iff key-within-block <= col-within-staircase
    jj = np.arange(128)[:, None]
    xx = np.arange(128)[None, :]
    st = (jj <= xx).astype(np.float64)
    stair = np.stack([st, st], axis=1)  # [128, 2, 128]
    return cs4, sn4, stair


def make_core_inputs(x, Wqkv, Wproj, core):
    """Build the per-core input map (numpy arrays, device dtypes)."""
    b, g = core // 2, core % 2
    np_x = _np_dt(DT_X)
    np_pv = _np_dt(DT_PV)
    np_o = _np_dt(DT_O)

    xT = np.ascontiguousarray(x[b].T)  # [C, T]
    xt4 = np.ascontiguousarray(
        xT.reshape(KT, 128, NCH, CH).transpose(2, 0, 1, 3)
    ).astype(np_x)

    Wq = Wqkv[g * 512 : (g + 1) * 512]
    Wk = Wqkv[C + g * 512 : C + (g + 1) * 512]
    Wv = Wqkv[2 * C + g * 512 : 2 * C + (g + 1) * 512]
    wqkT = np.vstack([Wq, Wk]).T  # [C, 1024]
    wqk = np.ascontiguousarray(wqkT.reshape(KT, 128, 1024)).astype(np_x)
    wvT = Wv.T  # [C, 512]
    wv = np.ascontiguousarray(wvT.reshape(KT, 128, 512)).astype(np_x)
    wpT = Wproj[:, g * 512 : (g + 1) * 512].T  # [512, C]
    wp = np.ascontiguousarray(wpT.reshape(NP, 128, C)).astype(np_o)

    cs4, sn4, stair = _host_tables()
    return {
        "xt4": xt4,
        "wqk": wqk,
        "wv": wv,
        "wp": wp,
        "cs4": cs4,
        "sn4": sn4,
        "stair": stair.astype(np_pv),
    }


LAST_RESULTS = None
_NC_CACHE = None


def kernel(x, Wqkv, Wproj):
    global LAST_RESULTS, _NC_CACHE
    from concourse.bass_utils import run_bass_kernel_spmd

    x = np.asarray(x, dtype=np.float32)
    Wqkv = np.asarray(Wqkv, dtype=np.float32)
    Wproj = np.asarray(Wproj, dtype=np.float32)

    if _NC_CACHE is None:
        _NC_CACHE = build_nc()
    nc = _NC_CACHE
    in_maps = [make_core_inputs(x, Wqkv, Wproj, core) for core in range(NCORES)]
    res = run_bass_kernel_spmd(nc, in_maps, list(range(NCORES)))
    LAST_RESULTS = res

    out = np.empty((B, T, C), dtype=np.float32)
    for b in range(B):
        out[b] = res.results[2 * b]["y"].astype(np.float32) + res.results[
            2 * b + 1
        ]["y"].astype(np.float32)
    return out


# revision 42
# speedup vs baseline: 1.0510x; 1.0201x over previous
"""Causal self-attention (RoPE) Trainium2 Bass kernel.

Problem: B=4, T=2048, C=1024, H=16 heads, D=64, fp32 I/O.
Sharding: 8 cores = 4 (batch) x 2 (head-group TP). Each core computes
qkv/attention/proj for 1 batch and 8 heads, producing a partial
projection output; the host sums the two TP partials per batch.

Per-core pipeline (chunk-interleaved to overlap PE matmuls with the
ACT-engine exp of the softmax):
  stage A (per 512-token chunk c): qkv projection + RoPE -> q_rot(c)
    (transient), k_rot[:, c] (persistent), v(c) (persistent, with an
    appended ones column for the softmax denominator)
  stage B (per chunk c, head-pair p, key-block jb): one [128, 1024]
    psum tile holds both heads' S_T scores for key block jb; the two
    K=64 QK matmuls are adjacent with different row groups so they
    run concurrently in the PE array.  One exp on ACT covers both
    heads (2-range AP when the causal io-shrink applies), a [128,128]
    staircase mask on DVE for diagonal blocks, then per-head
    out_T = v_ext^T @ P_T accumulated over key blocks; row 64 of the
    accumulator is the softmax denominator l.  Normalize with
    reciprocal (read straight from PSUM) + gpsimd partition_broadcast.
  proj (per chunk): y[chunk] = o_T^T @ WprojT, partial over this
    core's 512 input features.
"""

import numpy as np
from contextlib import ExitStack

import concourse.bacc as bacc
import concourse.bass as bass
import concourse.mybir as mybir
import concourse.tile as tile

# ---------------- constants ----------------
B = 4
T = 2048
C = 1024
H = 16
D = 64
L = 8  # local heads per core
NCORES = 8
ROPE_BASE = 10000.0

CH = 512  # t-chunk size
NCH = T // CH  # 4 chunks
KT = C // 128  # 8 contraction tiles
NP = L // 2  # 4 head-pair tiles
SCALE = 1.0 / np.sqrt(D)

F32 = mybir.dt.float32
BF16 = mybir.dt.bfloat16

# matmul operand dtypes
DT_X = BF16  # x / Wqkv / Wv operands
DT_K = BF16  # q_rot / k_rot
DT_PV = BF16  # P tiles, v tiles, masks
DT_O = BF16  # o_T tiles / WprojT


def _np_dt(dt):
    return mybir.dt.np(dt)


# ---------------- device kernel ----------------


def attn_body(ctx: ExitStack, tc: tile.TileContext, outs, ins):
    """outs = (y [T, C] f32,); ins = (xt4, wqk, wv, wp, cs4, sn4, stair)."""
    nc = tc.nc
    (y,) = outs if isinstance(outs, (tuple, list)) else (outs,)
    xt4, wqk, wv, wp, cs4, sn4, stair = ins

    TB = T // 128  # 16 key blocks

    consts = ctx.enter_context(tc.tile_pool(name="consts", bufs=1))
    xpool = ctx.enter_context(tc.tile_pool(name="xpool", bufs=16))
    cspool = ctx.enter_context(tc.tile_pool(name="cspool", bufs=4))
    qrpool = ctx.enter_context(tc.tile_pool(name="qrpool", bufs=8))
    rtmp = ctx.enter_context(tc.tile_pool(name="rtmp", bufs=4))
    ptpool = ctx.enter_context(tc.tile_pool(name="ptpool", bufs=6))
    otpool = ctx.enter_context(tc.tile_pool(name="otpool", bufs=16))
    yepool = ctx.enter_context(tc.tile_pool(name="yepool", bufs=3))
    lpool = ctx.enter_context(tc.tile_pool(name="lpool", bufs=4))
    pmisc = ctx.enter_context(tc.tile_pool(name="pmisc", bufs=2, space="PSUM"))
    pss_pool = ctx.enter_context(tc.tile_pool(name="pss", bufs=2, space="PSUM"))
    pso_pool = ctx.enter_context(tc.tile_pool(name="pso", bufs=2, space="PSUM"))

    # persistent tiles (wq/wk split so the startup q-group only waits on wq)
    wq_sb = [consts.tile([128, 512], DT_X, name=f"wq{k}") for k in range(KT)]
    wk_sb = [consts.tile([128, 512], DT_X, name=f"wk{k}") for k in range(KT)]
    wv_sb = [consts.tile([128, 512], DT_X, name=f"wv{k}") for k in range(KT)]
    wp_sb = [consts.tile([128, C], DT_O, name=f"wp{p}") for p in range(NP)]
    stair_sb = consts.tile([128, 2, 128], DT_PV, name="stair")
    k_rot = [consts.tile([128, T], DT_K, name=f"krot{p}") for p in range(NP)]
    v_sb = consts.tile([128, TB, L, 65], DT_PV, name="vsb")

    def load_first_chunk():
        # per-k (wq, xt) pairs stream first so the all-q startup group
        # ramps the PE densely with the DMA (warming the HAM clock-gate
        # early); wk then wv follow for the k and v groups
        xt_sb[0] = []
        for k in range(KT):
            nc.sync.dma_start(wq_sb[k][:], wqk[k, :, 0:512])
            xt = xpool.tile([128, CH], DT_X, name=f"xt0_{k}", tag="xt")
            nc.sync.dma_start(xt[:], xt4[0, k])
            xt_sb[0].append(xt)
            if k == 0:
                cs_sb[0] = cspool.tile([128, CH], DT_K, name="cs0", tag="cs")
                sn_sb[0] = cspool.tile([128, CH], DT_K, name="sn0", tag="sn")
                nc.sync.dma_start(cs_sb[0][:], cs4[0])
                nc.sync.dma_start(sn_sb[0][:], sn4[0])
            if k == 1:
                nc.sync.dma_start(stair_sb[:], stair[:])
        for k in range(KT):
            nc.sync.dma_start(wk_sb[k][:], wqk[k, :, 512:1024])
        for k in range(KT):
            nc.sync.dma_start(wv_sb[k][:], wv[k])
        # softmax-denominator ones column
        nc.vector.memset(v_sb[:, :, :, 64:65], 1.0)

    def load_consts_late():
        for p in range(NP):
            nc.sync.dma_start(wp_sb[p][:], wp[p])

    # per-chunk transient state
    xt_sb = {}
    q_rot = {}
    cs_sb = {}
    sn_sb = {}
    ot_sb = {}

    def load_chunk_inputs(c):
        us = []

        def mk_load(c):
            def f():
                cs_sb[c] = cspool.tile([128, CH], DT_K, name=f"cs{c}", tag="cs")
                sn_sb[c] = cspool.tile([128, CH], DT_K, name=f"sn{c}", tag="sn")
                nc.sync.dma_start(cs_sb[c][:], cs4[c])
                nc.sync.dma_start(sn_sb[c][:], sn4[c])
                xt_sb[c] = []
                for k in range(KT):
                    xt = xpool.tile([128, CH], DT_X, name=f"xt{c}_{k}", tag="xt")
                    nc.sync.dma_start(xt[:], xt4[c, k])
                    xt_sb[c].append(xt)

            return f

        us.append((mk_load(c), "pe"))
        return us

    def rope_tile(c, jt, ps, on_act):
        """RoPE for one [128, CH] q/k feature tile whose raw values sit in
        psum `ps`.  rot = raw*cos + swap(raw)*sin_signed.

        evac: psum -> bf16 q_sb (ACT engine when it is idle, else DVE);
        4 small swap copies (bf16, 4x mode); two TT mults + one TT add.
        """
        sn = sn_sb[c]
        q_sb = rtmp.tile([128, CH], DT_K, name=f"qsb{c}_{jt}", tag="qsb")
        if on_act:
            nc.scalar.copy(q_sb[:], ps[:])
        else:
            nc.vector.tensor_copy(q_sb[:], ps[:])
        qsw = rtmp.tile([128, CH], DT_K, name=f"qsw{c}_{jt}", tag="qsw")
        for blk in range(2):
            b0 = blk * 64
            nc.vector.tensor_copy(qsw[b0 : b0 + 32, :], q_sb[b0 + 32 : b0 + 64, :])
            nc.vector.tensor_copy(qsw[b0 + 32 : b0 + 64, :], q_sb[b0 : b0 + 32, :])
        qtmp = rtmp.tile([128, CH], DT_K, name=f"qtmp{c}_{jt}", tag="qtmp")
        nc.vector.tensor_tensor(
            out=qtmp[:], in0=qsw[:], in1=sn[:], op=mybir.AluOpType.mult
        )
        qraw = rtmp.tile([128, CH], DT_K, name=f"qraw{c}_{jt}", tag="qraw")
        nc.vector.tensor_tensor(
            out=qraw[:], in0=q_sb[:], in1=cs_sb[c][:], op=mybir.AluOpType.mult
        )
        if jt < NP:  # q tile
            dst = qrpool.tile([128, CH], DT_K, name=f"qrot{c}_{jt}", tag="qr")
            q_rot[(c, jt)] = dst
            nc.vector.tensor_tensor(
                out=dst[:], in0=qraw[:], in1=qtmp[:], op=mybir.AluOpType.add
            )
        else:  # k tile
            p = jt - NP
            nc.vector.tensor_tensor(
                out=k_rot[p][:, c * CH : (c + 1) * CH],
                in0=qraw[:],
                in1=qtmp[:],
                op=mybir.AluOpType.add,
            )

    def v_evac(c, tbl, ps, on_act):
        tb = c * 4 + tbl
        dst = v_sb[:, tb, :, 0:64]
        src = ps[:].rearrange("p (h d) -> p h d", h=L)
        if on_act:
            nc.scalar.copy(dst, src)
        else:
            nc.vector.tensor_copy(dst, src)

    def stage_a0_units():
        """Chunk 0 stage A with k-outer matmul groups of 4 units so the PE
        ramps while the first DMAs stream in; psum evacuations go to the
        idle ACT engine."""
        units = []
        groups = [[0, 1, 2, 3], [4, 5, 6, 7], [8, 9, 10, 11]]  # 8..11 = v tbl 0..3

        def mk_group(gi, group):
            def f():
                pss = {}
                for i, u in enumerate(group):
                    pool = pmisc if i < 2 else pss_pool
                    shape = [128, CH] if i < 2 else [128, 2 * CH]
                    tag = "pA" if i < 2 else "pss"
                    pss[u] = pool.tile(shape, F32, name=f"psA0g{gi}_{u}", tag=tag)
                for k in range(KT):
                    for u in group:
                        ps = pss[u][:, 0:CH]
                        if u < 8:  # q/k feature tile
                            w = wq_sb[k] if u < 4 else wk_sb[k]
                            nc.tensor.matmul(
                                ps,
                                w[:, (u % 4) * 128 : (u % 4 + 1) * 128],
                                xt_sb[0][k][:],
                                start=(k == 0),
                                stop=(k == KT - 1),
                            )
                        else:  # v token block
                            tbl = u - 8
                            nc.tensor.matmul(
                                ps,
                                xt_sb[0][k][:, tbl * 128 : (tbl + 1) * 128],
                                wv_sb[k][:],
                                start=(k == 0),
                                stop=(k == KT - 1),
                            )
                for u in group:
                    if u < 8:
                        rope_tile(0, u, pss[u][:, 0:CH], on_act=True)
                    else:
                        v_evac(0, u - 8, pss[u][:, 0:CH], on_act=True)

            return f

        for gi, g in enumerate(groups):
            units.append(mk_group(gi, g))
        return units

    def stage_a_units(c):
        """8 q/k feature tiles + 4 v t-blocks for chunk c, each split into
        two fill sub-units (4 contraction matmuls each) so interleaved
        fill blobs stay small."""
        units = []
        ps_live = {}

        def mk_qk(c, jt, half):
            def f():
                if half == 0:
                    ps_live[jt] = pmisc.tile(
                        [128, CH], F32, name=f"psA{c}_{jt}", tag="pA"
                    )
                ps = ps_live[jt]
                for k in range(half * 4, half * 4 + 4):
                    w = wq_sb[k] if jt < 4 else wk_sb[k]
                    nc.tensor.matmul(
                        ps[:],
                        w[:, (jt % 4) * 128 : (jt % 4 + 1) * 128],
                        xt_sb[c][k][:],
                        start=(k == 0),
                        stop=(k == KT - 1),
                    )
                if half == 1:
                    rope_tile(c, jt, ps_live.pop(jt), on_act=False)

            return f

        def mk_v(c, tbl, half):
            def f():
                if half == 0:
                    ps_live[8 + tbl] = pmisc.tile(
                        [128, CH], F32, name=f"psV{c}_{tbl}", tag="pA"
                    )
                ps = ps_live[8 + tbl]
                for k in range(half * 4, half * 4 + 4):
                    nc.tensor.matmul(
                        ps[:],
                        xt_sb[c][k][:, tbl * 128 : (tbl + 1) * 128],
                        wv_sb[k][:],
                        start=(k == 0),
                        stop=(k == KT - 1),
                    )
                if half == 1:
                    v_evac(c, tbl, ps_live.pop(8 + tbl), on_act=False)

            return f

        for jt in range(2 * NP):
            units.append((mk_qk(c, jt, 0), "pe"))
            units.append((mk_qk(c, jt, 1), "dve"))
        for tbl in range(4):
            units.append((mk_v(c, tbl, 0), "pe"))
            units.append((mk_v(c, tbl, 1), "dve"))
        return units

    def stage_b_units(c):
        """per chunk c: 4 head-pairs x (4c+4) key blocks, software-pipelined.

        Sub-unit (p, jb).qk: one [128, 2*CH] psum tile = both heads' S_T
        scores for key block jb (head a cols 0:CH, head b cols CH:2CH).
        The two K=64 QK matmuls are adjacent and land in different PE row
        groups (partitions 0:64 vs 64:128) so they stream concurrently.
        One exp on ACT covers both heads; diagonal blocks shrink to
        columns >= io and get a staircase mask on 128 cols.

        Sub-unit (p, jb).pv consumes the exp'd tile.  Emission runs the
        pv of block jb AFTER the qk of block jb+1 so the PE has covering
        work while the ACT exp is in flight.
        """
        units = []
        nblk = 4 * c + 4

        def mk_qk(c, p, jb):
            def f():
                qt = q_rot[(c, p)]
                kt_ = k_rot[p]
                io = (jb - 4 * c) * 128 if jb >= 4 * c else 0
                pss = pss_pool.tile(
                    [128, 2 * CH], F32, name=f"pss{c}_{p}_{jb}", tag="pss"
                )
                pss_unit[(p, jb)] = pss
                for idx, rb in ((0, 0), (1, 64)):
                    nc.tensor.matmul(
                        pss[:, idx * CH + io : (idx + 1) * CH],
                        kt_[rb : rb + 64, jb * 128 : (jb + 1) * 128],
                        qt[rb : rb + 64, io:],
                        start=True,
                        stop=True,
                    )
                pt = ptpool.tile([128, 2 * CH], DT_PV, name=f"pt{c}_{p}_{jb}", tag="pt")
                pt_unit[(p, jb)] = pt
                if io == 0:
                    nc.scalar.activation(
                        pt[:],
                        pss[:],
                        mybir.ActivationFunctionType.Exp,
                        scale=float(SCALE),
                    )
                else:
                    src = pss[:].rearrange("q (h t) -> q h t", h=2)[:, :, io:]
                    dst = pt[:].rearrange("q (h t) -> q h t", h=2)[:, :, io:]
                    nc.scalar.activation(
                        dst, src, mybir.ActivationFunctionType.Exp, scale=float(SCALE)
                    )
                if jb >= 4 * c:  # diagonal block -> staircase mask on 128 cols
                    pv = pt[:].rearrange("q (h t) -> q h t", h=2)[:, :, io : io + 128]
                    nc.vector.tensor_tensor(
                        out=pv, in0=pv, in1=stair_sb[:], op=mybir.AluOpType.mult
                    )

            return f

        def mk_pv(c, p, jb):
            def f():
                pt = pt_unit.pop((p, jb))
                io = (jb - 4 * c) * 128 if jb >= 4 * c else 0
                for idx, h in ((0, 2 * p), (1, 2 * p + 1)):
                    if jb == 0:
                        pso_unit[(c, h)] = pso_pool.tile(
                            [65, CH], F32, name=f"pso{c}_{h}", tag="pso"
                        )
                    pso = pso_unit[(c, h)]
                    nc.tensor.matmul(
                        pso[:, io:],
                        v_sb[:, jb, h, 0:65],
                        pt[:, idx * CH + io : (idx + 1) * CH],
                        start=(jb == 0),
                        stop=(jb == nblk - 1),
                    )

            return f

        def mk_norm(c, p, idx):
            def f():
                if idx == 0:
                    ot_sb[(c, p)] = otpool.tile(
                        [128, CH], DT_O, name=f"ot{c}_{p}", tag="ot"
                    )
                ot = ot_sb[(c, p)]
                h = 2 * p + idx
                pso = pso_unit[(c, h)]
                lsb = lpool.tile([1, CH], F32, name=f"lsb{c}_{h}", tag="lsb")
                nc.vector.tensor_copy(lsb[:], pso[64:65, :])
                linv = lpool.tile([1, CH], F32, name=f"linv{c}_{h}", tag="linv")
                nc.vector.reciprocal_approx_fast(linv[:], lsb[:])
                lb = lpool.tile([64, CH], F32, name=f"lb{c}_{h}", tag="lb")
                nc.gpsimd.partition_broadcast(lb[:], linv[:])
                nc.vector.tensor_tensor(
                    out=ot[idx * 64 : (idx + 1) * 64, :],
                    in0=pso[0:64, :],
                    in1=lb[:],
                    op=mybir.AluOpType.mult,
                )

            return f

        pso_unit = {}
        pss_unit = {}
        pt_unit = {}
        oraw_sb = {}
        lsb_sb = {}
        # software pipeline across the whole chunk (pairs run back-to-back):
        # qk(i+1) is emitted before pv(i) so exp latency is covered.  Units
        # are (fn, fill_weight); norms get extra fill to cover the
        # reciprocal/broadcast chain that gates the next pair's PV.
        from collections import deque

        seq = [(p, jb) for p in range(NP) for jb in range(nblk)]
        DEPTH = 2  # qk runs this many units ahead of pv
        norm_q = deque()

        def push_norm():
            if norm_q:
                units.append((norm_q.popleft(), 4, True))

        def push_pv(j):
            units.append((mk_pv(c, *seq[j]), 1, False))
            if seq[j][1] == nblk - 1:  # pair done -> queue both heads' norms
                norm_q.append(mk_norm(c, seq[j][0], 0))
                norm_q.append(mk_norm(c, seq[j][0], 1))

        for i, s in enumerate(seq):
            w = 3 if i < 2 else 1  # front-load fill to cover the previous
            units.append((mk_qk(c, *s), w, False))  # chunk's trailing evacs
            push_norm()
            if i - DEPTH >= 0:
                push_pv(i - DEPTH)
            push_norm()
        for j in range(len(seq) - DEPTH, len(seq)):
            push_pv(j)
            push_norm()
        while norm_q:
            push_norm()
        return units

    def proj_units(c, evac_act=False):
        """Both oc halves per unit: consecutive matmuls share the ot lhsT
        so only one weight load is paid per contraction step."""
        units = []

        def mk_proj(c, tbl):
            def f():
                pss = [
                    pmisc.tile([128, CH], F32, name=f"psY{c}_{tbl}_{oc}", tag="pA")
                    for oc in range(2)
                ]
                for p in range(NP):
                    for oc in range(2):
                        nc.tensor.matmul(
                            pss[oc][:],
                            ot_sb[(c, p)][:, tbl * 128 : (tbl + 1) * 128],
                            wp_sb[p][:, oc * CH : (oc + 1) * CH],
                            start=(p == 0),
                            stop=(p == NP - 1),
                        )
                for oc in range(2):
                    ye = yepool.tile(
                        [128, CH], DT_O, name=f"ye{c}_{tbl}_{oc}", tag="ye"
                    )
                    if evac_act:
                        nc.scalar.copy(ye[:], pss[oc][:])
                    else:
                        nc.vector.tensor_copy(ye[:], pss[oc][:])
                    nc.sync.dma_start(
                        y[c * CH + tbl * 128 : c * CH + (tbl + 1) * 128,
                          oc * CH : (oc + 1) * CH],
                        ye[:],
                    )

            return f

        for tbl in range(4):
            units.append((mk_proj(c, tbl), "dve"))
        return units

    def emit_interleaved(primary, secondary, reserve=0.10):
        """Emit weighted primary units (fn, w, pe_only) with secondary
        units (fn, kind) spread between them proportionally to weight.
        At pe_only slots (norm boundaries) prefer pure-matmul fill so no
        extra DVE work queues ahead of the norm chain; hold back a
        fraction of secondary for the phase tail."""
        if not primary:
            for u, _ in secondary:
                u()
            return
        nres = int(len(secondary) * reserve)
        spread = spread_l = secondary[: len(secondary) - nres]
        ns = len(spread)
        emitted = [False] * ns
        ndone = 0

        def pull(pe_only):
            nonlocal ndone
            idx = None
            skipped = 0
            for j in range(ns):
                if emitted[j]:
                    continue
                if not pe_only or spread[j][1] == "pe":
                    idx = j
                    break
                skipped += 1
                if skipped > 3:
                    break
            if idx is None:
                for j in range(ns):
                    if not emitted[j]:
                        idx = j
                        break
            if idx is None:
                return False
            emitted[idx] = True
            spread[idx][0]()
            ndone += 1
            return True

        wtot = sum(w for _, w, _ in primary)
        cum = 0
        for u, w, pe_only in primary:
            u()
            cum += w
            want = cum * ns // wtot
            while ndone < want:
                if not pull(pe_only):
                    break
        for j in range(ns):
            if not emitted[j]:
                spread[j][0]()
        for u, _ in secondary[len(secondary) - nres :]:
            u()

    # ---- emission ----
    load_first_chunk()
    for u in stage_a0_units():
        u()
    load_consts_late()
    for c in range(NCH):
        fill = []
        if c + 1 < NCH:
            fill += load_chunk_inputs(c + 1)
            fill += stage_a_units(c + 1)
        if c == NCH - 1:
            # all deferred projections fill the ACT-bound final chunk
            for cc in range(NCH - 1):
                fill += proj_units(cc)
        emit_interleaved(stage_b_units(c), fill)
    for u, _ in proj_units(NCH - 1, evac_act=True):
        u()


def build_nc():
    nc = bacc.Bacc("TRN2", target_bir_lowering=False, debug=False)
    xt4 = nc.declare_dram_parameter("xt4", [NCH, KT, 128, CH], DT_X, isOutput=False)
    wqk = nc.declare_dram_parameter("wqk", [KT, 128, 1024], DT_X, isOutput=False)
    wv = nc.declare_dram_parameter("wv", [KT, 128, 512], DT_X, isOutput=False)
    wp = nc.declare_dram_parameter("wp", [NP, 128, C], DT_O, isOutput=False)
    cs4 = nc.declare_dram_parameter("cs4", [NCH, 128, CH], DT_K, isOutput=False)
    sn4 = nc.declare_dram_parameter("sn4", [NCH, 128, CH], DT_K, isOutput=False)
    stair = nc.declare_dram_parameter("stair", [128, 2, 128], DT_PV, isOutput=False)
    yout = nc.declare_dram_parameter("y", [T, C], DT_O, isOutput=True)

    with tile.TileContext(nc) as tc:
        with ExitStack() as ctx:
            attn_body(
                ctx, tc, (yout[:],),
                (xt4[:], wqk[:], wv[:], wp[:], cs4[:], sn4[:], stair[:]),
            )
    nc.compile()
    return nc


# ---------------- host side ----------------


def _rope_tables_np():
    inv_freq = 1.0 / (ROPE_BASE ** (np.arange(0, D, 2, dtype=np.float64) / D))
    t = np.arange(T, dtype=np.float64)
    freqs = np.outer(t, inv_freq)  # [T, 32]
    emb = np.concatenate([freqs, freqs], axis=-1)  # [T, 64]
    return np.cos(emb), np.sin(emb)  # [T, 64] each


def _host_tables():
    cos, sin = _rope_tables_np()  # [T, 64]
    d_of_r = np.arange(128) % 64
    cs = cos[:, d_of_r].T.astype(np.float32)  # [128, T]
    sn_abs = sin[:, d_of_r].T
    sign = np.where((d_of_r % 64) < 32, -1.0, 1.0)[:, None]
    sn = (sn_abs * sign).astype(np.float32)  # [128, T]
    np_k = _np_dt(DT_K)
    cs4 = np.ascontiguousarray(cs.reshape(128, NCH, CH).transpose(1, 0, 2)).astype(np_k)
    sn4 = np.ascontiguousarray(sn.reshape(128, NCH, CH).transpose(1, 0, 2)).astype(np_k)

    # universal diagonal staircase [128 key rows, 2 (head copies), 128 cols]:
    # valid iff key-within-block <= col-within-staircase
    jj = np.arange(128)[:, None]
    xx = np.arange(128)[None, :]
    st = (jj <= xx).astype(np.float64)
    stair = np.stack([st, st], axis=1)  # [128, 2, 128]
    return cs4, sn4, stair


def make_core_inputs(x, Wqkv, Wproj, core):
    """Build the per-core input map (numpy arrays, device dtypes)."""
    b, g = core // 2, core % 2
    np_x = _np_dt(DT_X)
    np_pv = _np_dt(DT_PV)
    np_o = _np_dt(DT_O)

    xT = np.ascontiguousarray(x[b].T)  # [C, T]
    xt4 = np.ascontiguousarray(
        xT.reshape(KT, 128, NCH, CH).transpose(2, 0, 1, 3)
    ).astype(np_x)

    Wq = Wqkv[g * 512 : (g + 1) * 512]
    Wk = Wqkv[C + g * 512 : C + (g + 1) * 512]
    Wv = Wqkv[2 * C + g * 512 : 2 * C + (g + 1) * 512]
    wqkT = np.vstack([Wq, Wk]).T  # [C, 1024]
    wqk = np.ascontiguousarray(wqkT.reshape(KT, 128, 1024)).astype(np_x)
    wvT = Wv.T  # [C, 512]
    wv = np.ascontiguousarray(wvT.reshape(KT, 128, 512)).astype(np_x)
    wpT = Wproj[:, g * 512 : (g + 1) * 512].T  # [512, C]
    wp = np.ascontiguousarray(wpT.reshape(NP, 128, C)).astype(np_o)

    cs4, sn4, stair = _host_tables()
    return {
        "xt4": xt4,
        "wqk": wqk,
        "wv": wv,
        "wp": wp,
        "cs4": cs4,
        "sn4": sn4,
        "stair": stair.astype(np_pv),
    }


LAST_RESULTS = None
_NC_CACHE = None


def kernel(x, Wqkv, Wproj):
    global LAST_RESULTS, _NC_CACHE
    from concourse.bass_utils import run_bass_kernel_spmd

    x = np.asarray(x, dtype=np.float32)
    Wqkv = np.asarray(Wqkv, dtype=np.float32)
    Wproj = np.asarray(Wproj, dtype=np.float32)

    if _NC_CACHE is None:
        _NC_CACHE = build_nc()
    nc = _NC_CACHE
    in_maps = [make_core_inputs(x, Wqkv, Wproj, core) for core in range(NCORES)]
    res = run_bass_kernel_spmd(nc, in_maps, list(range(NCORES)))
    LAST_RESULTS = res

    out = np.empty((B, T, C), dtype=np.float32)
    for b in range(B):
        out[b] = res.results[2 * b]["y"].astype(np.float32) + res.results[
            2 * b + 1
        ]["y"].astype(np.float32)
    return out
